# revision 8
# baseline (speedup 1.0000x reference)
"""HSS linear attention Trainium2 kernel.

Strategy: data-parallel over batch (8 cores x 1 batch element). Per core the
causal HSS scan is computed exactly with hardware prefix scans
(tensor_tensor_scan) over rank-space product streams, in feature-major layout:

  x^T (PE transposes) -> qkT/vT projections (+elu1) -> constant-matrix
  permutation matmuls build rank^2 product streams -> DVE prefix scans
  (exact reference summation order) -> apply/reduce/expand matmuls -> output
  projection with fused 1/denom scaling.

All permutations / UL/UR expansions are folded into host-precomputed constant
matrices shipped as replicated DRAM inputs.
"""
import numpy as np

import concourse.bacc as bacc
import concourse.mybir as mybir
import concourse.tile as tile
from concourse import bass_utils
from concourse.masks import make_identity

D_MODEL, D_HEAD, T = 1024, 64, 2048
B = 8
BLOCK_SIZES = [4, 8, 16, 32]
RANKS = [4, 8, 8, 8]
NUM_NODES = [8, 4, 2, 1]
OFF88 = [0, 32, 64, 80]
TB = 512
NB = T // TB
F32 = mybir.dt.float32

# level tiles: (name, level, delta list). l3 shares its scan tile with z.
TILES = [
    ("l0", 0, [0, 1, 2, 3]),
    ("l1a", 1, [0, 1, 2, 3]),
    ("l1b", 1, [4, 5, 6, 7]),
    ("l2", 2, [0, 1, 2, 3, 4, 5, 6, 7]),
    ("l3", 3, [0, 1, 2, 3, 4, 5, 6, 7]),
]
REP_FAM = {"l0": "l0", "l1a": "l1", "l1b": "l1", "l2": "l2", "l3": "l3"}


def _idx88(l, n, r):
    return OFF88[l] + n * RANKS[l] + r


def _tile_rows(l, dlist):
    return [(d, n, r) for d in dlist for n in range(NUM_NODES[l])
            for r in range(RANKS[l])]


def build_constants(inp):
    UL = [np.asarray(inp[f"UL{l}"], np.float32) for l in range(4)]
    VR = [np.asarray(inp[f"VR{l}"], np.float32) for l in range(4)]
    UR = [np.asarray(inp[f"UR{l}"], np.float32) for l in range(4)]
    VL = [np.asarray(inp[f"VL{l}"], np.float32) for l in range(4)]
    c = {}
    c["Wqk"] = np.ascontiguousarray(
        np.concatenate([inp["Wq"], inp["Wk"]], axis=1), dtype=np.float32)
    c["Wv"] = np.ascontiguousarray(inp["Wv"], dtype=np.float32)
    c["Wo"] = np.ascontiguousarray(inp["Wo"], dtype=np.float32)
    PuL = np.zeros((64, 88), np.float32)
    PvR = np.zeros((64, 88), np.float32)
    Pvlx = np.zeros((64, 88), np.float32)
    for l in range(4):
        bs, rl = BLOCK_SIZES[l], RANKS[l]
        for n in range(NUM_NODES[l]):
            top = slice(n * 2 * bs, n * 2 * bs + bs)
            bot = slice(n * 2 * bs + bs, (n + 1) * 2 * bs)
            cc = slice(_idx88(l, n, 0), _idx88(l, n, 0) + rl)
            PuL[top, cc] = UL[l]
            PvR[bot, cc] = VR[l]
            Pvlx[top, cc] = VL[l]
    c["PuL"], c["PvR"], c["Pvlx"] = PuL, PvR, Pvlx

    def mshift(d):
        m = np.zeros((64, 64), np.float32)
        for n in range(16):
            for i in range(4):
                m[4 * n + (i + d) % 4, 4 * n + i] = 1.0
        return m

    c["Mvs0"] = np.concatenate([mshift(0), mshift(1)], axis=1)
    c["Mvs1"] = np.concatenate([mshift(2), mshift(3)], axis=1)
    c["Mk2"] = np.concatenate([np.eye(64, dtype=np.float32)] * 2, axis=1)
    c["Cleaf"] = np.concatenate([np.eye(64, dtype=np.float32)] * 2, axis=0)
    for name, l, dlist in TILES:
        rows = _tile_rows(l, dlist)
        nr = len(rows)
        rl, bs = RANKS[l], BLOCK_SIZES[l]
        Rin1 = np.zeros((88, nr), np.float32)
        Sin2 = np.zeros((88, nr), np.float32)
        Ctop = np.zeros((nr, 64), np.float32)
        Cbot = np.zeros((nr, 64), np.float32)
        for j, (d, n, r) in enumerate(rows):
            rp = (r + d) % rl
            Rin1[_idx88(l, n, r), j] = 1.0
            Sin2[_idx88(l, n, rp), j] = 1.0
            Ctop[j, n * 2 * bs:n * 2 * bs + bs] = UL[l][:, r]
            Cbot[j, n * 2 * bs + bs:(n + 1) * 2 * bs] = UR[l][:, rp]
        c[f"Rin1_{name}"] = Rin1
        c[f"Sin2_{name}"] = Sin2
        c[f"Ctop_{name}"] = Ctop
        c[f"Cbot_{name}"] = Cbot
    # l1a/l1b share the same rep builder; drop the duplicate.
    del c["Rin1_l1b"]
    c["ones64"] = np.ones((64, 1), np.float32)
    c["ones1"] = np.ones((1, 1), np.float32)
    return {k: np.ascontiguousarray(v, dtype=np.float32) for k, v in c.items()}


def build_bass():
    nc = bacc.Bacc("TRN2", target_bir_lowering=False, debug=False)
    AD = mybir.AluOpType.add
    MU = mybir.AluOpType.mult

    x = nc.dram_tensor("x", [T, D_MODEL], F32, kind="ExternalInput").ap()
    out = nc.dram_tensor("out", [T, D_MODEL], F32, kind="ExternalOutput").ap()

    cshapes = {
        "Wqk": (D_MODEL, 128), "Wv": (D_MODEL, 64), "Wo": (64, D_MODEL),
        "PuL": (64, 88), "PvR": (64, 88), "Pvlx": (64, 88),
        "Mvs0": (64, 128), "Mvs1": (64, 128), "Mk2": (64, 128),
        "Cleaf": (128, 64), "ones64": (64, 1), "ones1": (1, 1),
        "Rin1_l0": (88, 128), "Rin1_l1a": (88, 128), "Rin1_l2": (88, 128),
        "Rin1_l3": (88, 64),
    }
    for name, l, dlist in TILES:
        nr = len(_tile_rows(l, dlist))
        cshapes[f"Sin2_{name}"] = (88, nr)
        cshapes[f"Ctop_{name}"] = (nr, 64)
        cshapes[f"Cbot_{name}"] = (nr, 64)
    ctop64 = {"Ctop_l3", "Cbot_l3"}
    cdram = {k: nc.dram_tensor(k, list(v), F32, kind="ExternalInput").ap()
             for k, v in cshapes.items()}

    with tile.TileContext(nc) as tc:
        with (
            tc.tile_pool(name="const", bufs=1) as cp,
            tc.tile_pool(name="xa", bufs=5) as xap,
            tc.tile_pool(name="xt", bufs=9) as xtp,
            tc.tile_pool(name="sb", bufs=2) as sp,
            tc.tile_pool(name="st", bufs=2) as stp,
            tc.tile_pool(name="ob", bufs=3) as obp,
            tc.tile_pool(name="ps_tp", bufs=2, space="PSUM") as ps_tp,
            tc.tile_pool(name="ps_qk", bufs=1, space="PSUM") as ps_qk,
            tc.tile_pool(name="ps_vv", bufs=1, space="PSUM") as ps_vv,
            tc.tile_pool(name="ps_bb", bufs=3, space="PSUM") as ps_bb,
            tc.tile_pool(name="ps_yy", bufs=1, space="PSUM") as ps_yy,
        ):
            # ---- constants to SBUF ----
            csb = {}
            for k, shp in cshapes.items():
                if k in ("Wqk", "Wv"):
                    continue
                if k in ctop64:
                    tl = cp.tile([128, shp[1]], F32, tag=f"c_{k}")
                    nc.sync.dma_start(tl[64:128, :], cdram[k])
                    csb[k] = tl[64:128, :]
                else:
                    tl = cp.tile(list(shp), F32, tag=f"c_{k}")
                    nc.sync.dma_start(tl, cdram[k])
                    csb[k] = tl
            # Wqk/Wv as per-chunk stationary tiles (contraction on partitions)
            wqk = cp.tile([128, 8, 128], F32, tag="c_Wqk")
            wv = cp.tile([128, 8, 64], F32, tag="c_Wv")
            for d in range(8):
                nc.sync.dma_start(wqk[:, d, :],
                                  cdram["Wqk"][d * 128:(d + 1) * 128, :])
                nc.sync.dma_start(wv[:, d, :],
                                  cdram["Wv"][d * 128:(d + 1) * 128, :])
            ident = cp.tile([128, 128], F32, tag="ident")
            make_identity(nc, ident)
            zeros = cp.tile([128, TB], F32, tag="zeros")
            nc.vector.memset(zeros, 0.0)

            carry = {}

            def scan(name, S, P, rows):
                init = 0.0 if name not in carry else carry[name][:, 0:1]
                nc.vector.tensor_tensor_scan(
                    S[:rows, :], P[:rows, :], zeros[:rows, :], init, AD, AD)
                ctile = sp.tile([rows, 1], F32, tag=f"carry_{name}")
                nc.vector.tensor_copy(ctile, S[:rows, TB - 1:TB])
                carry[name] = ctile

            for b in range(NB):
                t0 = b * TB
                # ---- load x, build xT chunks via PE transpose ----
                xa = []
                for i in range(4):
                    tl = xap.tile([128, D_MODEL], F32, tag="xa")
                    nc.sync.dma_start(tl, x[t0 + i * 128: t0 + (i + 1) * 128, :])
                    xa.append(tl)
                xts = []
                for d in range(8):
                    tp = ps_tp.tile([128, TB], F32, tag="tp")
                    for i in range(4):
                        nc.tensor.transpose(
                            tp[:, i * 128:(i + 1) * 128],
                            xa[i][:, d * 128:(d + 1) * 128], ident)
                    xd = xtp.tile([128, TB], F32, tag="xT")
                    nc.scalar.copy(xd, tp)
                    xts.append(xd)
                # ---- projections ----
                qk_ps = ps_qk.tile([128, TB], F32, tag="qk")
                v_ps = ps_vv.tile([64, TB], F32, tag="vv")
                for d in range(8):
                    nc.tensor.matmul(qk_ps, wqk[:, d, :], xts[d],
                                     start=(d == 0), stop=(d == 7))
                for d in range(8):
                    nc.tensor.matmul(v_ps, wv[:, d, :], xts[d],
                                     start=(d == 0), stop=(d == 7))
                # ---- elu1 ----
                mn = sp.tile([128, TB], F32, tag="mn")
                nc.vector.tensor_scalar_min(mn, qk_ps, 0.0)
                ex = sp.tile([128, TB], F32, tag="ex")
                nc.scalar.activation(ex, mn, mybir.ActivationFunctionType.Exp)
                rl_ = sp.tile([128, TB], F32, tag="rl")
                nc.scalar.activation(rl_, qk_ps,
                                     mybir.ActivationFunctionType.Relu)
                qkT = stp.tile([128, TB], F32, tag="qkT")
                nc.vector.tensor_add(qkT, ex, rl_)
                vT = stp.tile([64, TB], F32, tag="vT")
                nc.scalar.copy(vT, v_ps)
                qT = qkT[0:64, :]
                kTs = stp.tile([64, TB], F32, tag="kTs")
                nc.sync.dma_start(kTs, qkT[64:128, :])

                # ---- feature-major rank projections (product side) ----
                def mm_to_sbuf(lhsT, rhs, m, tag, eng="act"):
                    ps = ps_bb.tile([m, TB], F32, tag="bb")
                    nc.tensor.matmul(ps, lhsT, rhs, start=True, stop=True)
                    sb = sp.tile([m, TB], F32, tag=tag)
                    if eng == "act":
                        nc.scalar.copy(sb, ps)
                    else:
                        nc.vector.tensor_copy(sb, ps)
                    return sb

                uLT = mm_to_sbuf(csb["PuL"], kTs, 88, "uLT")
                vRT = mm_to_sbuf(csb["PvR"], vT, 88, "vRT")

                # ---- leaf products ----
                k2 = mm_to_sbuf(csb["Mk2"], kTs, 128, "k2")
                Pf = []
                for p, mv in enumerate(("Mvs0", "Mvs1")):
                    vs_ps = ps_bb.tile([128, TB], F32, tag="bb")
                    nc.tensor.matmul(vs_ps, csb[mv], vT, start=True, stop=True)
                    pf = stp.tile([128, TB], F32, tag="PL", bufs=4)
                    nc.vector.tensor_tensor(pf, k2, vs_ps, MU)
                    Pf.append(pf)
                # ---- level products ----
                reps = {}
                for fam, cname in (("l0", "Rin1_l0"), ("l1", "Rin1_l1a"),
                                   ("l2", "Rin1_l2")):
                    reps[fam] = mm_to_sbuf(csb[cname], uLT, 128, "rep")
                # l3 rep lives on partitions 64-127 (z occupies 0-63)
                rep3_ps = ps_bb.tile([128, TB], F32, tag="bb")
                nc.tensor.matmul(rep3_ps[64:128, :], csb["Rin1_l3"], uLT,
                                 start=True, stop=True)
                rep3 = sp.tile([128, TB], F32, tag="rep")
                nc.scalar.copy(rep3[64:128, :], rep3_ps[64:128, :])
                reps["l3"] = rep3

                PL = {}
                for name, l, dlist in TILES:
                    nr = len(_tile_rows(l, dlist))
                    if name != "l3":
                        shf_ps = ps_bb.tile([nr, TB], F32, tag="bb")
                        nc.tensor.matmul(shf_ps, csb[f"Sin2_{name}"], vRT,
                                         start=True, stop=True)
                        pl = stp.tile([nr, TB], F32, tag="PL", bufs=4)
                        nc.vector.tensor_tensor(pl, reps[name if name in reps
                                                else REP_FAM[name]][:nr, :],
                                                shf_ps, MU)
                        PL[name] = pl
                # l3z tile: rows 0-63 = kT (z stream), rows 64-127 = l3 products
                shf3_ps = ps_bb.tile([128, TB], F32, tag="bb")
                nc.tensor.matmul(shf3_ps[64:128, :], csb["Sin2_l3"], vRT,
                                 start=True, stop=True)
                plz = stp.tile([128, TB], F32, tag="PL", bufs=4)
                nc.vector.tensor_tensor(plz[64:128, :], rep3[64:128, :],
                                        shf3_ps[64:128, :], MU)
                nc.scalar.copy(plz[0:64, :], kTs)
                PL["l3z"] = plz

                # ---- scans ----
                Sf = []
                for p in range(2):
                    s = stp.tile([128, TB], F32, tag="S", bufs=4)
                    scan(f"f{p}", s, Pf[p], 128)
                    Sf.append(s)
                SL = {}
                for name, l, dlist in TILES:
                    if name == "l3":
                        continue
                    nr = len(_tile_rows(l, dlist))
                    s = stp.tile([nr, TB], F32, tag="S", bufs=4)
                    scan(name, s, PL[name], nr)
                    SL[name] = s
                sz = stp.tile([128, TB], F32, tag="SZ")
                scan("l3z", sz, PL["l3z"], 128)

                # ---- apply: query-side projections ----
                vrxT = mm_to_sbuf(csb["PvR"], qT, 88, "vrxT")
                vlxT = mm_to_sbuf(csb["Pvlx"], qT, 88, "vlxT")

                yT_ps = ps_yy.tile([64, TB], F32, tag="yy")
                acc = [0]

                def y_acc(lhsT, rhs):
                    nc.tensor.matmul(yT_ps, lhsT, rhs, start=(acc[0] == 0),
                                     stop=(acc[0] == 11))
                    acc[0] += 1

                for p, mv in enumerate(("Mvs0", "Mvs1")):
                    qs_ps = ps_bb.tile([128, TB], F32, tag="bb")
                    nc.tensor.matmul(qs_ps, csb[mv], qT, start=True, stop=True)
                    ap = sp.tile([128, TB], F32, tag="ap", bufs=3)
                    nc.vector.tensor_tensor(ap, Sf[p], qs_ps, MU)
                    y_acc(csb["Cleaf"], ap)
                for name, l, dlist in TILES:
                    nr = len(_tile_rows(l, dlist))
                    if name == "l3":
                        stile = sz[64:128, :]
                        off = 64
                    else:
                        stile = SL[name]
                        off = 0
                    vrxs_ps = ps_bb.tile([off + nr, TB], F32, tag="bb")
                    nc.tensor.matmul(vrxs_ps[off:off + nr, :],
                                     csb[f"Sin2_{name}"], vrxT,
                                     start=True, stop=True)
                    atop = sp.tile([off + nr, TB], F32, tag="ap", bufs=3)
                    nc.vector.tensor_tensor(atop[off:off + nr, :], stile,
                                            vrxs_ps[off:off + nr, :], MU)
                    y_acc(csb[f"Ctop_{name}"], atop[off:off + nr, :])
                    vlxr_ps = ps_bb.tile([off + nr, TB], F32, tag="bb")
                    rname = "Rin1_l1a" if REP_FAM[name] == "l1" else \
                        f"Rin1_{REP_FAM[name]}"
                    nc.tensor.matmul(vlxr_ps[off:off + nr, :], csb[rname],
                                     vlxT, start=True, stop=True)
                    abot = sp.tile([off + nr, TB], F32, tag="ap", bufs=3)
                    nc.vector.tensor_tensor(abot[off:off + nr, :], stile,
                                            vlxr_ps[off:off + nr, :], MU)
                    y_acc(csb[f"Cbot_{name}"], abot[off:off + nr, :])
                assert acc[0] == 12

                # ---- denominator ----
                dm = sp.tile([64, TB], F32, tag="dm")
                nc.vector.tensor_tensor(dm, sz[0:64, :], qT, MU)
                den_ps = ps_bb.tile([1, TB], F32, tag="bb")
                nc.tensor.matmul(den_ps, csb["ones64"], dm, start=True,
                                 stop=True)
                den = sp.tile([1, TB], F32, tag="den")
                nc.vector.tensor_scalar_max(den, den_ps, 1e-6)
                rec = sp.tile([1, TB], F32, tag="rec")
                nc.vector.reciprocal(rec, den)

                yT = sp.tile([64, TB], F32, tag="yT")
                nc.scalar.copy(yT, yT_ps)

                # ---- output projection + divide ----
                for i in range(4):
                    rec_ps = ps_bb.tile([128, 1], F32, tag="bb")
                    nc.tensor.matmul(rec_ps, rec[0:1, i * 128:(i + 1) * 128],
                                     csb["ones1"], start=True, stop=True)
                    recT = sp.tile([128, 1], F32, tag="recT")
                    nc.vector.tensor_copy(recT, rec_ps)
                    ob = obp.tile([128, D_MODEL], F32, tag="ob")
                    for h in range(2):
                        op_ps = ps_bb.tile([128, 512], F32, tag="bb")
                        nc.tensor.matmul(op_ps, yT[:, i * 128:(i + 1) * 128],
                                         csb["Wo"][:, h * 512:(h + 1) * 512],
                                         start=True, stop=True)
                        if h == 0:
                            nc.scalar.activation(
                                ob[:, h * 512:(h + 1) * 512], op_ps,
                                mybir.ActivationFunctionType.Copy,
                                scale=recT[:, 0:1])
                        else:
                            nc.vector.tensor_scalar_mul(
                                ob[:, h * 512:(h + 1) * 512], op_ps,
                                recT[:, 0:1])
                    nc.sync.dma_start(
                        out[t0 + i * 128: t0 + (i + 1) * 128, :], ob)
    nc.compile()
    return nc


_NC = None


def kernel(**inputs):
    global _NC
    if _NC is None:
        _NC = build_bass()
    consts = build_constants(inputs)
    x = np.ascontiguousarray(np.asarray(inputs["x"], np.float32))
    in_maps = [dict(consts, x=x[b]) for b in range(B)]
    res = bass_utils.run_bass_kernel_spmd(_NC, in_maps,
                                          core_ids=list(range(B)))
    return np.stack([res.results[b]["out"] for b in range(B)], axis=0)
